# revision 1
# baseline (speedup 1.0000x reference)
"""MoE layer (top-2 of 8 experts) on 8 Trainium2 NeuronCores.

Strategy (expert-parallel, per the sharding hint):
  * Host computes the (tiny) gating network: probs = softmax(x @ w_gate),
    top-2 experts + normalized gates per token.  This is the sharding
    decision — it determines how tokens are dispatched to cores.
  * Tokens are dispatched by expert id: core e receives exactly the tokens
    routed to expert e (padded to a common capacity C), plus W1[e], W2[e]
    in bf16.  Each core runs the expert FFN  o = relu(x @ W1e) @ W2e  as a
    Bass/Tile kernel (bf16 matmuls, fp32 accumulation).
  * Capacity cap: C is the smallest feasible block sum >= max expert load,
    except when rounding DOWN sheds <= MAX_DROP assignments — then the
    lowest-gate assignments of overloaded experts are dropped (standard
    capacity-factor MoE dropping; error contribution is tiny).
  * Host combines: y[n] = sum_k gate[n,k] * o_{e(n,k)}[slot(n,k)].

Device kernel layout (per core, SPMD over 8 cores):
  inputs  xT [D, C] bf16 (tokens transposed), w1 [D, F] bf16, w2 [F, D] bf16
  output  out [C, D] bf16
  mm1: hT[f,:]  = w1[:,f].T @ xT      (f on PSUM partitions, tokens moving)
  relu -> h_sb bf16
  mm2: out[t,:] += h_sb[:,t].T @ w2[fc,:]  (tokens on PSUM partitions,
       d moving, accumulated over all F in PSUM)
All weights stay resident in SBUF (16 MB bf16); tokens are processed in
blocks of up to TB=384 (the last block may be 128/256 to trim capacity)
so the mm2 accumulators fit in PSUM.
"""

import time

import numpy as np
import ml_dtypes

import concourse.bass as bass
import concourse.mybir as mybir
import concourse.tile as tile
from concourse import bacc
from concourse.bass_utils import run_bass_kernel_spmd

N, D, F, E, TOPK = 8192, 1024, 4096, 8, 2
P = 128
TB = 384          # max tokens per block (3 PSUM m-tiles; 6 accum banks + 2 pipe)
MAX_DROP = 8      # max low-gate assignments to shed when trimming capacity
NCORES = 8

BF16 = mybir.dt.bfloat16
F32 = mybir.dt.float32

_program_cache: dict[int, "bass.Bass"] = {}
LAST_RESULTS = None    # BassKernelResults of the most recent run (for test.py)
TRACE = False          # test.py can flip this before calling kernel()


def _block_sizes(C: int) -> list[int]:
    """C = 384*a + r with r in {0, 128, 256}; blocks of 384 plus one ragged."""
    assert C % P == 0 and C > 0
    a, r = divmod(C, TB)
    assert r in (0, 128, 256), (C, r)
    return [TB] * a + ([r] if r else [])


def capacity_for(max_load: int) -> int:
    """Smallest feasible C >= max_load, or the next lower feasible C when
    that sheds <= MAX_DROP assignments."""
    c_up = -(-max_load // P) * P
    c_dn = c_up - P
    if c_dn >= TB and max_load - c_dn <= MAX_DROP:
        return c_dn
    return max(TB, c_up)


def _build_program(C: int, bench_iters: int = 1) -> "bass.Bass":
    """One expert FFN: out[C, D] = relu(x @ W1) @ W2 with x given transposed.

    bench_iters > 1 wraps the compute in a hardware loop (same result, run
    repeatedly) so test harnesses can measure steady-state HW time from the
    wall-clock delta between two iteration counts."""
    KD = D // P            # 8  k-tiles over d_model
    KF = F // P            # 32 f-chunks of 128
    ND = D // 512          # 2  output n-tiles of 512
    QF = 4                 # weight chunks (pipelined load)
    KFQ = KF // QF         # 8 f-chunks per weight chunk
    blocks = _block_sizes(C)
    offs = np.concatenate([[0], np.cumsum(blocks)]).tolist()

    nc = bacc.Bacc("TRN2", target_bir_lowering=False, debug=False,
                   num_devices=NCORES)
    xT = nc.dram_tensor("xT", [D, C], BF16, kind="ExternalInput")
    w1 = nc.dram_tensor("w1", [D, F], BF16, kind="ExternalInput")
    w2 = nc.dram_tensor("w2", [F, D], BF16, kind="ExternalInput")
    out = nc.dram_tensor("out", [C, D], BF16, kind="ExternalOutput")

    xT_r = xT[:].rearrange("(ko p) n -> ko p n", p=P)
    w1_r = w1[:].rearrange("(ko p) f -> ko p f", p=P)
    w2_r = w2[:].rearrange("(ko p) d -> ko p d", p=P)

    with tile.TileContext(nc) as tc:
        with (
            tc.tile_pool(name="wpool", bufs=1) as wpool,
            tc.tile_pool(name="hpool", bufs=6) as hpool,
            tc.tile_pool(name="opool", bufs=2) as opool,
            tc.tile_pool(name="ph_pool", bufs=2, space="PSUM") as ph_pool,
            tc.tile_pool(name="po_pool", bufs=1, space="PSUM") as po_pool,
        ):
            xT_sb = wpool.tile([P, KD, C], BF16, name="xT_sb")
            for k in range(KD):
                nc.sync.dma_start(xT_sb[:, k, :], xT_r[k])

            # weights quartered along F so compute starts after 1/4 is loaded
            w1q = [wpool.tile([P, KD, KFQ * P], BF16, name=f"w1q{q}") for q in range(QF)]
            w2q = [wpool.tile([P, KFQ, D], BF16, name=f"w2q{q}") for q in range(QF)]
            for q in range(QF):
                for k in range(KD):
                    nc.sync.dma_start(
                        w1q[q][:, k, :], w1_r[k][:, q * KFQ * P:(q + 1) * KFQ * P]
                    )
                for k in range(KFQ):
                    nc.sync.dma_start(w2q[q][:, k, :], w2_r[q * KFQ + k])

            def token_block(t):
                tb = blocks[t]
                o0 = offs[t]
                tmb = tb // P
                out_blk = out[:][o0:o0 + tb, :].rearrange(
                    "(tm p) d -> p tm d", p=P)

                po = [
                    [
                        po_pool.tile([P, 512], F32, name=f"po_{tm}_{nd}",
                                     tag=f"po_{tm}_{nd}")
                        for nd in range(ND)
                    ]
                    for tm in range(tmb)
                ]

                def mm1(fc, t=t):
                    ph = ph_pool.tile([P, TB], F32, name="ph", tag="ph")
                    q, c = fc // KFQ, (fc % KFQ) * P
                    for ki in range(KD):
                        nc.tensor.matmul(
                            ph[:, :tb],
                            lhsT=w1q[q][:, ki, c:c + P],
                            rhs=xT_sb[:, ki, o0:o0 + tb],
                            start=(ki == 0),
                            stop=(ki == KD - 1),
                        )
                    h = hpool.tile([P, TB], BF16, name="h", tag="h")
                    nc.scalar.activation(h[:, :tb], ph[:, :tb],
                                         mybir.ActivationFunctionType.Relu)
                    return h

                # software pipeline: emit mm1(fc+1) before mm2(fc) so the PE
                # never waits on the relu of the h-tile it is about to consume
                h_cur = mm1(0)
                for fc in range(KF):
                    h_next = mm1(fc + 1) if fc + 1 < KF else None
                    for tm in range(tmb):
                        for nd in range(ND):
                            nc.tensor.matmul(
                                po[tm][nd],
                                lhsT=h_cur[:, tm * P:(tm + 1) * P],
                                rhs=w2q[fc // KFQ][:, fc % KFQ, nd * 512:(nd + 1) * 512],
                                start=(fc == 0),
                                stop=(fc == KF - 1),
                            )
                    h_cur = h_next

                o_sb = opool.tile([P, 3, D], BF16, name="o_sb", tag="o_sb")
                for tm in range(tmb):
                    for nd in range(ND):
                        nc.vector.tensor_copy(
                            o_sb[:, tm, nd * 512:(nd + 1) * 512], po[tm][nd]
                        )
                nc.sync.dma_start(out_blk, o_sb[:, :tmb, :])

            if bench_iters > 1:
                with tc.For_i(0, bench_iters, 1):
                    for t in range(len(blocks)):
                        token_block(t)
            else:
                for t in range(len(blocks)):
                    token_block(t)
    nc.compile()
    return nc


def _gate_and_dispatch(x, w_gate):
    """Replicates the reference gating exactly (fp32): softmax + top-2."""
    logits = x.astype(np.float32) @ w_gate.astype(np.float32)        # [N, E]
    m = logits.max(-1, keepdims=True)
    p = np.exp(logits - m)
    probs = p / p.sum(-1, keepdims=True)
    # jax.lax.top_k: descending, ties broken by lower index -> stable argsort
    tk_idx = np.argsort(-probs, axis=1, kind="stable")[:, :TOPK]
    tk_vals = np.take_along_axis(probs, tk_idx, axis=1)
    tk_gates = tk_vals / (tk_vals.sum(-1, keepdims=True) + 1e-9)
    return tk_idx, tk_gates


def kernel(x, w_gate, W1, W2):
    global LAST_RESULTS
    x = np.asarray(x, dtype=np.float32)
    w_gate = np.asarray(w_gate, dtype=np.float32)
    W1 = np.asarray(W1, dtype=np.float32)
    W2 = np.asarray(W2, dtype=np.float32)
    n_tok = x.shape[0]

    tk_idx, tk_gates = _gate_and_dispatch(x, w_gate)

    # flat assignment lists
    eid = tk_idx.reshape(-1).astype(np.int64)          # expert of assignment
    gat = tk_gates.reshape(-1).astype(np.float32)      # gate of assignment
    tok = np.repeat(np.arange(n_tok), TOPK)            # token of assignment

    loads = np.bincount(eid, minlength=E)
    C = capacity_for(int(loads.max()))

    # per-expert kept assignments (lowest-gate dropped when over capacity)
    kept_tok, kept_gate = [], []
    for e in range(E):
        sel = np.where(eid == e)[0]
        if len(sel) > C:
            keep = np.argsort(-gat[sel], kind="stable")[:C]
            sel = sel[np.sort(keep)]
        kept_tok.append(tok[sel])
        kept_gate.append(gat[sel])

    # per-core inputs: tokens for expert e, transposed and padded to C
    in_maps = []
    for e in range(E):
        idx = kept_tok[e]
        xe_T = np.zeros((D, C), dtype=ml_dtypes.bfloat16)
        xe_T[:, :len(idx)] = np.ascontiguousarray(x[idx].T).astype(ml_dtypes.bfloat16)
        in_maps.append({
            "xT": xe_T,
            "w1": np.ascontiguousarray(W1[e]).astype(ml_dtypes.bfloat16),
            "w2": np.ascontiguousarray(W2[e]).astype(ml_dtypes.bfloat16),
        })

    nc = _program_cache.get(C)
    if nc is None:
        nc = _build_program(C)
        _program_cache[C] = nc

    try:
        res = run_bass_kernel_spmd(nc, in_maps, core_ids=list(range(NCORES)),
                                   trace=TRACE)
    except Exception:
        # transient NRT/device hiccups (e.g. NRT_EXEC_UNIT_UNRECOVERABLE)
        # have been observed to clear after a short wait — retry once
        time.sleep(20)
        res = run_bass_kernel_spmd(nc, in_maps, core_ids=list(range(NCORES)),
                                   trace=TRACE)
    LAST_RESULTS = res

    # combine: y[n] = sum over kept assignments gate * out_expert[slot]
    y = np.zeros((n_tok, D), np.float32)
    for e in range(E):
        o = np.asarray(res.results[e]["out"]).astype(np.float32)
        nk = len(kept_tok[e])
        # token indices are unique within one expert (top-k experts are
        # distinct per token), so fancy-index += is safe here
        y[kept_tok[e]] += kept_gate[e][:, None] * o[:nk]
    return y



# revision 2
# speedup vs baseline: 1.0892x; 1.0892x over previous
"""MoE layer (top-2 of 8 experts) on 8 Trainium2 NeuronCores.

Strategy (expert-parallel along the *F axis* — "global F-split"):
  * Host computes the (tiny) gating network: probs = softmax(x @ w_gate),
    top-2 experts + normalized gates per token.
  * The expert FFN decomposes along the hidden axis F:
        o = relu(x @ W1) @ W2 = sum_fslices relu(x @ W1[:, fs]) @ W2[fs, :]
    so core c is given the f-slice [c*F/8, (c+1)*F/8) of EVERY expert's
    W1/W2 (16.8 MB bf16 — same footprint as one whole expert) and computes
    the partial output of EVERY assignment over its slice.  Per-core work
    is exactly sum_e load_e * F/8 = N*K*F/8 — perfectly balanced across
    cores regardless of expert load skew, with zero dropped tokens.
  * All cores run the SAME program on the SAME dispatched-token stream
    (assignments grouped by expert); only the weight slices differ.
  * Host combines: o = sum_cores o_partial;  y[n] = sum_k gate[n,k]*o[slot].

Device kernel layout (per core, SPMD over 8 cores):
  inputs  xT [D, A]    bf16  dispatched tokens, transposed, expert-grouped
          w1 [E, D, FS] bf16  this core's f-slice of every expert's W1
          w2 [E, FS, D] bf16  this core's f-slice of every expert's W2
  output  out [A, D]   bf16  partial (f-slice) expert outputs per assignment
  Per expert segment, tokens are processed in blocks of up to TB=384:
    mm1: ph[f,:]   = w1[:,f].T @ xT_blk      (f on PSUM partitions)
    relu -> h bf16 (scalar engine)
    mm2: po[t,:]  += h[:,t].T @ w2[fc,:]     (tokens on PSUM partitions,
         accumulated over the FC=4 local f-chunks)
  Weights stay resident in SBUF; x is streamed per block (double-buffered);
  PSUM->SBUF output copies are split across Vector and Scalar engines so
  the PE never waits on an accumulator bank.
"""

import time

import numpy as np
import ml_dtypes

import concourse.bass as bass
import concourse.mybir as mybir
import concourse.tile as tile
from concourse import bacc
from concourse.bass_utils import run_bass_kernel_spmd

N, D, F, E, TOPK = 8192, 1024, 4096, 8, 2
P = 128
NCORES = 8
FS = F // NCORES   # 512: f-slice width per core
FC = FS // P       # 4 local f-chunks of 128
KD = D // P        # 8 k-tiles over d_model
ND = D // 512      # 2 output n-tiles of 512
TB = 384           # max tokens per block (3 PSUM m-tiles; 6 accum banks + 2 pipe)

BF16 = mybir.dt.bfloat16
F32 = mybir.dt.float32

_program_cache: dict[tuple, "bass.Bass"] = {}
LAST_RESULTS = None    # BassKernelResults of the most recent run (for test.py)
TRACE = False          # test.py can flip this before calling kernel()


def _blocks_of(load: int) -> list[int]:
    a, r = divmod(load, TB)
    return [TB] * a + ([r] if r else [])


def _build_program(loads: tuple, bench_iters: int = 1) -> "bass.Bass":
    """Partial FFN over this core's f-slice for all E experts:
    out[A, D] = concat_e relu(x_e @ W1e[:, fs]) @ W2e[fs, :]
    where the token stream xT is grouped by expert with segment lengths
    `loads` (A = sum(loads)).

    bench_iters > 1 wraps the compute in a hardware loop (same result, run
    repeatedly) so test harnesses can measure steady-state HW time from the
    wall-clock delta between two iteration counts."""
    A = int(sum(loads))
    seg_off = np.concatenate([[0], np.cumsum(loads)]).astype(int).tolist()

    nc = bacc.Bacc("TRN2", target_bir_lowering=False, debug=False,
                   num_devices=NCORES)
    xT = nc.dram_tensor("xT", [D, A], BF16, kind="ExternalInput")
    w1 = nc.dram_tensor("w1", [E, D, FS], BF16, kind="ExternalInput")
    w2 = nc.dram_tensor("w2", [E, FS, D], BF16, kind="ExternalInput")
    out = nc.dram_tensor("out", [A, D], BF16, kind="ExternalOutput")

    xT_r = xT[:].rearrange("(ko p) n -> ko p n", p=P)
    w1_r = w1[:].rearrange("e (ko p) f -> e ko p f", p=P)
    w2_r = w2[:].rearrange("e (fc p) d -> e fc p d", p=P)

    with tile.TileContext(nc) as tc:
        with (
            tc.tile_pool(name="wpool", bufs=1) as wpool,
            tc.tile_pool(name="xpool", bufs=3) as xpool,
            tc.tile_pool(name="hpool", bufs=4) as hpool,
            tc.tile_pool(name="opool", bufs=2) as opool,
            tc.tile_pool(name="ph_pool", bufs=2, space="PSUM") as ph_pool,
            tc.tile_pool(name="po_pool", bufs=1, space="PSUM") as po_pool,
        ):
            w1_sb = wpool.tile([P, E, KD, FS], BF16, name="w1_sb")
            w2_sb = wpool.tile([P, E, FC, D], BF16, name="w2_sb")
            for e in range(E):
                for k in range(KD):
                    nc.sync.dma_start(w1_sb[:, e, k, :], w1_r[e, k])
                for c in range(FC):
                    nc.sync.dma_start(w2_sb[:, e, c, :], w2_r[e, c])

            def token_block(e, o0, tb):
                tmb = -(-tb // P)

                xb = xpool.tile([P, KD, TB], BF16, name="xb", tag="xb")
                for k in range(KD):
                    nc.sync.dma_start(xb[:, k, :tb], xT_r[k][:, o0:o0 + tb])

                po = [
                    [
                        po_pool.tile([P, 512], F32, name=f"po_{tm}_{nd}",
                                     tag=f"po_{tm}_{nd}")
                        for nd in range(ND)
                    ]
                    for tm in range(tmb)
                ]

                def mm1(fc):
                    ph = ph_pool.tile([P, TB], F32, name="ph", tag="ph")
                    for ki in range(KD):
                        nc.tensor.matmul(
                            ph[:, :tb],
                            lhsT=w1_sb[:, e, ki, fc * P:(fc + 1) * P],
                            rhs=xb[:, ki, :tb],
                            start=(ki == 0),
                            stop=(ki == KD - 1),
                        )
                    h = hpool.tile([P, TB], BF16, name="h", tag="h")
                    nc.scalar.activation(h[:, :tb], ph[:, :tb],
                                         mybir.ActivationFunctionType.Relu)
                    return h

                # software pipeline: emit mm1(fc+1) before mm2(fc) so the PE
                # never waits on the relu of the h-tile it is about to consume
                h_cur = mm1(0)
                for fc in range(FC):
                    h_next = mm1(fc + 1) if fc + 1 < FC else None
                    for tm in range(tmb):
                        cnt = min(P, tb - tm * P)
                        for nd in range(ND):
                            nc.tensor.matmul(
                                po[tm][nd][:cnt, :],
                                lhsT=h_cur[:, tm * P:tm * P + cnt],
                                rhs=w2_sb[:, e, fc, nd * 512:(nd + 1) * 512],
                                start=(fc == 0),
                                stop=(fc == FC - 1),
                            )
                    h_cur = h_next

                # retire accumulators, alternating engines so neither the
                # vector nor the scalar queue becomes the PE's bottleneck
                o_sb = opool.tile([P, 3, D], BF16, name="o_sb", tag="o_sb")
                for i, (tm, nd) in enumerate(
                        [(t, n) for t in range(tmb) for n in range(ND)]):
                    cnt = min(P, tb - tm * P)
                    dst = o_sb[:cnt, tm, nd * 512:(nd + 1) * 512]
                    if i % 2 == 0:
                        nc.vector.tensor_copy(dst, po[tm][nd][:cnt, :])
                    else:
                        nc.scalar.activation(
                            dst, po[tm][nd][:cnt, :],
                            mybir.ActivationFunctionType.Copy)
                for tm in range(tmb):
                    cnt = min(P, tb - tm * P)
                    nc.sync.dma_start(out[:][o0 + tm * P:o0 + tm * P + cnt, :],
                                      o_sb[:cnt, tm, :])

            def body():
                for e in range(E):
                    o0 = seg_off[e]
                    for tb in _blocks_of(int(loads[e])):
                        token_block(e, o0, tb)
                        o0 += tb

            if bench_iters > 1:
                with tc.For_i(0, bench_iters, 1):
                    body()
            else:
                body()
    nc.compile()
    return nc


def _gate_and_dispatch(x, w_gate):
    """Replicates the reference gating exactly (fp32): softmax + top-2."""
    logits = x.astype(np.float32) @ w_gate.astype(np.float32)        # [N, E]
    m = logits.max(-1, keepdims=True)
    p = np.exp(logits - m)
    probs = p / p.sum(-1, keepdims=True)
    # jax.lax.top_k: descending, ties broken by lower index -> stable argsort
    tk_idx = np.argsort(-probs, axis=1, kind="stable")[:, :TOPK]
    tk_vals = np.take_along_axis(probs, tk_idx, axis=1)
    tk_gates = tk_vals / (tk_vals.sum(-1, keepdims=True) + 1e-9)
    return tk_idx, tk_gates


def kernel(x, w_gate, W1, W2):
    global LAST_RESULTS
    x = np.asarray(x, dtype=np.float32)
    w_gate = np.asarray(w_gate, dtype=np.float32)
    W1 = np.asarray(W1, dtype=np.float32)
    W2 = np.asarray(W2, dtype=np.float32)
    n_tok = x.shape[0]

    tk_idx, tk_gates = _gate_and_dispatch(x, w_gate)

    # flat assignment lists, grouped by expert (stable within expert)
    eid = tk_idx.reshape(-1).astype(np.int64)          # expert of assignment
    gat = tk_gates.reshape(-1).astype(np.float32)      # gate of assignment
    tok = np.repeat(np.arange(n_tok), TOPK)            # token of assignment
    order = np.argsort(eid, kind="stable")
    tok_d, gat_d = tok[order], gat[order]
    loads = tuple(int(v) for v in np.bincount(eid, minlength=E))

    # dispatched token stream, transposed: [D, A] — identical on all cores
    xT_d = np.ascontiguousarray(x[tok_d].T).astype(ml_dtypes.bfloat16)

    # per-core inputs: the core's f-slice of every expert's weights
    in_maps = []
    for c in range(NCORES):
        fsl = slice(c * FS, (c + 1) * FS)
        in_maps.append({
            "xT": xT_d,
            "w1": np.ascontiguousarray(W1[:, :, fsl]).astype(ml_dtypes.bfloat16),
            "w2": np.ascontiguousarray(W2[:, fsl, :]).astype(ml_dtypes.bfloat16),
        })

    nc = _program_cache.get(loads)
    if nc is None:
        nc = _build_program(loads)
        _program_cache[loads] = nc

    try:
        res = run_bass_kernel_spmd(nc, in_maps, core_ids=list(range(NCORES)),
                                   trace=TRACE)
    except Exception:
        # transient NRT/device hiccups (e.g. NRT_EXEC_UNIT_UNRECOVERABLE)
        # have been observed to clear after a short wait — retry once
        time.sleep(20)
        res = run_bass_kernel_spmd(nc, in_maps, core_ids=list(range(NCORES)),
                                   trace=TRACE)
    LAST_RESULTS = res

    # combine: sum the per-core f-slice partials, then gate-weighted scatter
    o_full = np.zeros((len(tok_d), D), np.float32)
    for c in range(NCORES):
        o_full += np.asarray(res.results[c]["out"]).astype(np.float32)

    y = np.zeros((n_tok, D), np.float32)
    off = 0
    for e in range(E):
        nk = loads[e]
        # token indices are unique within one expert (top-k experts are
        # distinct per token), so fancy-index += is safe here
        sl = slice(off, off + nk)
        y[tok_d[sl]] += gat_d[sl][:, None] * o_full[sl]
        off += nk
    return y
